# revision 1
# baseline (speedup 1.0000x reference)
"""Trainium2 Bass kernel for nn_MoE_81209241633272.

MoE layer: 16 experts, top-4 routing, gated-SiLU expert MLPs (2048->1024->2048)
plus an always-on shared gated MLP (2048->2048->2048), over 4096 tokens.

Strategy (expert-parallel across 8 cores):
  - Each core computes 2 experts (dense over all tokens; router coef zeroes the
    non-selected tokens) plus a 1/8 column-slice of the shared MLP.
  - Activations live in transposed layout x^T [D, T] so every matmul uses the
    weights in their natural layout and chains without transposes:
        h^T[I,T]  = matmul(lhsT=w1[D,I] tiles, rhs=x^T)         (PSUM [128,512])
        y [T,D]   = matmul(lhsT=h^T tiles,     rhs=w2[I,D])     (PSUM [128,512])
  - Router: logits computed bit-accurately via split bf16 (hi/lo) operands:
        logits = gh@xh + (gl@xh + gh@xl)   (lo*lo term negligible)
    done as two accumulation groups into one PSUM [48, 512] using packed gate
    matrices, then transposed to [T,16] (tiny identity matmul) for softmax +
    top-4 thresholding. Per-core gate columns are permuted so this core's
    experts are always columns 0 and 1 (softmax/top-k are permutation
    invariant); the top-4 mask is computed on raw logits so expert selection
    is bit-identical across cores.
  - Expert weights stream per chunk as half-tensors (12 x 2MB loads cycling 6
    SBUF slots) so prefetch overlaps compute with fine granularity.
  - Per chunk of 512 tokens the partial y (2 experts + shared slice) is
    ReduceScatter'd (sum) across the 8 cores; host concatenates the slices.
    y traffic rides the Activation HWDGE queue so the collective-gated output
    copy never head-of-line-blocks weight prefetch (SP queue). The last
    chunk's reduce runs per token-slice to shrink the kernel tail.

All matmuls are bf16 inputs with fp32 PSUM accumulation; everything else
(softmax, coef, y accumulation) is fp32.
"""

import numpy as np
import ml_dtypes

import concourse.bass as bass
import concourse.bacc as bacc
import concourse.mybir as mybir
from concourse.tile import TileContext
from concourse.masks import make_identity

BF16 = ml_dtypes.bfloat16
F32 = np.float32

N_CORES = 8
P = 128
B, S = 4, 1024
T = B * S              # 4096 tokens
D = 2048               # model dim
E = 16                 # experts
TOP_K = 4
I_EXP = 1024           # expert inter dim
SH_INTER = 2048        # shared inter dim (total)
SH_PC = SH_INTER // N_CORES  # shared inter slice per core = 256

CH = 512               # tokens per chunk (PSUM free-dim limit for fp32)
KO = D // P            # 16 k-tiles over D
IEO = I_EXP // P       # 8 i-tiles per expert
IEH = IEO // 2         # 4 i-tiles per half weight
ISO = SH_PC // P       # 2 i-tiles for shared slice
DCH = 512              # output D chunk
NDCH = D // DCH        # 4
NTS = CH // P          # 4 token-slices per chunk
RS_OUT = CH // N_CORES  # 64 rows per core from each chunk's reduce-scatter

AX = mybir.AxisListType
ALU = mybir.AluOpType
ACT = mybir.ActivationFunctionType
dt = mybir.dt


def build_nc(n_chunks=T // CH):
    nc = bacc.Bacc("TRN2", target_bir_lowering=False, num_devices=N_CORES)

    # ---- kernel I/O (per-core tensors; host supplies core-specific data) ----
    xh_d = nc.dram_tensor("xh", [n_chunks, P, KO, CH], dt.bfloat16, kind="ExternalInput")
    xl_d = nc.dram_tensor("xl", [n_chunks, P, KO, CH], dt.bfloat16, kind="ExternalInput")
    w1a_d = nc.dram_tensor("w1a", [P, KO, I_EXP], dt.bfloat16, kind="ExternalInput")
    w3a_d = nc.dram_tensor("w3a", [P, KO, I_EXP], dt.bfloat16, kind="ExternalInput")
    w2a_d = nc.dram_tensor("w2a", [P, IEO, D], dt.bfloat16, kind="ExternalInput")
    w1b_d = nc.dram_tensor("w1b", [P, KO, I_EXP], dt.bfloat16, kind="ExternalInput")
    w3b_d = nc.dram_tensor("w3b", [P, KO, I_EXP], dt.bfloat16, kind="ExternalInput")
    w2b_d = nc.dram_tensor("w2b", [P, IEO, D], dt.bfloat16, kind="ExternalInput")
    ws13_d = nc.dram_tensor("ws13", [P, KO, 2 * SH_PC], dt.bfloat16, kind="ExternalInput")
    ws2_d = nc.dram_tensor("ws2", [P, ISO, D], dt.bfloat16, kind="ExternalInput")
    g1_d = nc.dram_tensor("g1", [P, KO, 3 * E], dt.bfloat16, kind="ExternalInput")
    g2_d = nc.dram_tensor("g2", [P, KO, 3 * E], dt.bfloat16, kind="ExternalInput")

    y_out = nc.dram_tensor("y_out", [n_chunks, RS_OUT, D], dt.float32, kind="ExternalOutput")

    # internal DRAM for the collective (collectives can't touch kernel I/O)
    y_part = nc.dram_tensor("y_part", [n_chunks, CH, D], dt.float32)
    y_rs = nc.dram_tensor("y_rs", [n_chunks, RS_OUT, D], dt.float32)

    with TileContext(nc) as tc:
        with (
            tc.tile_pool(name="const", bufs=1) as cpool,
            tc.tile_pool(name="xp", bufs=2) as xpool,
            tc.tile_pool(name="wp", bufs=4) as wpool,
            tc.tile_pool(name="hp", bufs=1) as hpool,
            tc.tile_pool(name="hsp", bufs=2) as hspool,
            tc.tile_pool(name="sp", bufs=3) as spool,
            tc.tile_pool(name="yp", bufs=3) as ypool,
            tc.tile_pool(name="gp", bufs=2) as gpool,
            tc.tile_pool(name="smp", bufs=2) as smpool,
            tc.tile_pool(name="php", bufs=3, space="PSUM") as php,
            tc.tile_pool(name="pgp", bufs=2, space="PSUM") as pgp,
            tc.tile_pool(name="pyp", bufs=3, space="PSUM") as pyp,
        ):
            # ---- small resident constants ----
            # identity re-emitted by the DVE so the transpose matmuls depend
            # on a single semaphore (LDW weight-loads only fit one sync wait)
            ident_g = cpool.tile([E, E], dt.float32, tag="ident_g")
            make_identity(nc, ident_g)
            ident = cpool.tile([E, E], dt.float32, tag="ident")
            nc.vector.tensor_copy(ident, ident_g)
            g1_sb = cpool.tile([P, KO, 3 * E], dt.bfloat16, tag="g1")
            nc.sync.dma_start(g1_sb, g1_d[:])
            g2_sb = cpool.tile([P, KO, 3 * E], dt.bfloat16, tag="g2")
            nc.sync.dma_start(g2_sb, g2_d[:])

            ws13_sb = None
            ws2_sb = None

            def wload(dram, mid, col0, ncols):
                w = wpool.tile([P, mid, ncols], dt.bfloat16, tag="w", name="w")
                nc.sync.dma_start(w, dram[:, :, col0:col0 + ncols])
                return w

            HWC = I_EXP // 2   # w1/w3 half width (512)
            HW2 = D // 2       # w2 half width (1024)

            for c in range(n_chunks):
                # ---- stream this chunk's activations ----
                xh_sb = xpool.tile([P, KO, CH], dt.bfloat16, tag="xh")
                nc.sync.dma_start(xh_sb, xh_d[c])
                xl_sb = xpool.tile([P, KO, CH], dt.bfloat16, tag="xl")
                nc.sync.dma_start(xl_sb, xl_d[c])
                if c == 0:
                    # shared-MLP weights load once, after the first chunk's x
                    # so the gate isn't stuck behind them on the DMA queue
                    ws13_sb = cpool.tile([P, KO, 2 * SH_PC], dt.bfloat16, tag="ws13")
                    nc.sync.dma_start(ws13_sb, ws13_d[:])
                    ws2_sb = cpool.tile([P, ISO, D], dt.bfloat16, tag="ws2")
                    nc.sync.dma_start(ws2_sb, ws2_d[:])

                # first six half-weight loads fill all slots: e0 + e1-first-half
                w1a0 = wload(w1a_d, KO, 0, HWC)
                w3a0 = wload(w3a_d, KO, 0, HWC)
                w1a1 = wload(w1a_d, KO, HWC, HWC)
                w3a1 = wload(w3a_d, KO, HWC, HWC)
                w1b0 = wload(w1b_d, KO, 0, HWC)
                w3b0 = wload(w3b_d, KO, 0, HWC)

                # ---- gate: logits^T [16, CH] via packed split-precision matmuls ----
                # pg rows 0:16 = gh@xh ; rows 32:48 = gl@xh + gh@xl (32-aligned base)
                pg = pgp.tile([48, CH], dt.float32, tag="pg")
                for ko in range(KO):
                    nc.tensor.matmul(pg, g1_sb[:, ko, :], xh_sb[:, ko, :],
                                     start=(ko == 0), stop=False)
                for ko in range(KO):
                    nc.tensor.matmul(pg, g2_sb[:, ko, :], xl_sb[:, ko, :],
                                     start=False, stop=(ko == KO - 1))
                logits_hi = gpool.tile([16, CH], dt.float32, tag="lgh")
                nc.vector.tensor_copy(logits_hi, pg[0:16, :])
                logits_sb = gpool.tile([16, CH], dt.float32, tag="lg")
                nc.vector.tensor_add(logits_sb, logits_hi, pg[32:48, :])

                # ---- softmax + exact top-4 per token-slice ----
                coef_c = gpool.tile([P, NTS, E], dt.float32, tag="coef")
                for t in range(NTS):
                    # transpose [16,128] -> [128,16] as a regular tiny matmul:
                    # out[m,n] = sum_k logits[k,m] * I16[k,n] = logits[n,m]
                    pt = pgp.tile([P, E], dt.float32, tag="pg")
                    nc.tensor.matmul(pt, logits_sb[:, t * P:(t + 1) * P],
                                     ident, start=True, stop=True)
                    # softmax (max-subtracted, fp32)
                    mx = smpool.tile([P, 1], dt.float32, tag="mx")
                    nc.vector.reduce_max(mx, pt, axis=AX.X)
                    nm = smpool.tile([P, 1], dt.float32, tag="nm")
                    nc.vector.tensor_scalar_mul(nm, mx, -1.0)
                    ex = smpool.tile([P, E], dt.float32, tag="ex")
                    ssum = smpool.tile([P, 1], dt.float32, tag="ss")
                    nc.scalar.activation(ex, pt, ACT.Exp, bias=nm, scale=1.0,
                                         accum_out=ssum)
                    rcp = smpool.tile([P, 1], dt.float32, tag="rc")
                    nc.vector.reciprocal(rcp, ssum)
                    probs = smpool.tile([P, E], dt.float32, tag="pr")
                    nc.vector.tensor_scalar_mul(probs, ex, rcp)
                    # 4th-largest logit as threshold (bit-identical across cores)
                    work = smpool.tile([P, E], dt.float32, tag="wk")
                    nc.vector.tensor_copy(work, pt)
                    for _ in range(TOP_K - 1):
                        m = smpool.tile([P, 1], dt.float32, tag="m")
                        nc.vector.reduce_max(m, work, axis=AX.X)
                        msk = smpool.tile([P, E], dt.float32, tag="msk")
                        nc.vector.tensor_scalar(msk, work, m, 1.0e4,
                                                op0=ALU.is_ge, op1=ALU.mult)
                        nc.vector.tensor_sub(work, work, msk)
                    m4 = smpool.tile([P, 1], dt.float32, tag="m4")
                    nc.vector.reduce_max(m4, work, axis=AX.X)
                    gem = smpool.tile([P, E], dt.float32, tag="gem")
                    nc.vector.tensor_scalar(gem, pt, m4, None, op0=ALU.is_ge)
                    nc.vector.tensor_mul(coef_c[:, t, :], probs, gem)

                # ---- shared-expert h (resident weights) ----
                hs = hspool.tile([P, ISO, CH], dt.bfloat16, tag="hs")
                for i in range(ISO):
                    p1 = php.tile([P, CH], dt.float32, tag="ph")
                    for ko in range(KO):
                        nc.tensor.matmul(p1, ws13_sb[:, ko, i * P:(i + 1) * P],
                                         xh_sb[:, ko, :],
                                         start=(ko == 0), stop=(ko == KO - 1))
                    p3 = php.tile([P, CH], dt.float32, tag="ph")
                    for ko in range(KO):
                        nc.tensor.matmul(p3, ws13_sb[:, ko, SH_PC + i * P:SH_PC + (i + 1) * P],
                                         xh_sb[:, ko, :],
                                         start=(ko == 0), stop=(ko == KO - 1))
                    sl = spool.tile([P, CH], dt.bfloat16, tag="sl")
                    nc.scalar.activation(sl, p1, ACT.Sigmoid)
                    nc.vector.tensor_mul(sl, sl, p1)
                    nc.vector.tensor_mul(hs[:, i, :], sl, p3)

                # ---- expert h phases (streamed half-weights) ----
                h_tiles = []
                w2a = w2b = None
                for ei in range(2):
                    w1h = (w1a0, w1a1) if ei == 0 else (w1b0, w1b1)
                    w3h = (w3a0, w3a1) if ei == 0 else (w3b0, w3b1)
                    he = hpool.tile([P, IEO, CH], dt.bfloat16, tag=f"h{ei}")
                    for i in range(IEO):
                        wi, off = (0, i) if i < IEH else (1, i - IEH)
                        p1 = php.tile([P, CH], dt.float32, tag="ph")
                        for ko in range(KO):
                            nc.tensor.matmul(p1, w1h[wi][:, ko, off * P:(off + 1) * P],
                                             xh_sb[:, ko, :],
                                             start=(ko == 0), stop=(ko == KO - 1))
                        p3 = php.tile([P, CH], dt.float32, tag="ph")
                        for ko in range(KO):
                            nc.tensor.matmul(p3, w3h[wi][:, ko, off * P:(off + 1) * P],
                                             xh_sb[:, ko, :],
                                             start=(ko == 0), stop=(ko == KO - 1))
                        sl = spool.tile([P, CH], dt.bfloat16, tag="sl")
                        nc.scalar.activation(sl, p1, ACT.Sigmoid)
                        nc.vector.tensor_mul(sl, sl, p1)
                        nc.vector.tensor_mul(he[:, i, :], sl, p3)
                    h_tiles.append(he)
                    if ei == 0:
                        # e1 second half + e0 down-projection into freed slots
                        w1b1 = wload(w1b_d, KO, HWC, HWC)
                        w3b1 = wload(w3b_d, KO, HWC, HWC)
                        w2a = (wload(w2a_d, IEO, 0, HW2), wload(w2a_d, IEO, HW2, HW2))
                h0, h1 = h_tiles
                w2b = (wload(w2b_d, IEO, 0, HW2), wload(w2b_d, IEO, HW2, HW2))

                # ---- phase 2: y[T,D] per (t, d) tile; combine with router coef ----
                for t in range(NTS):
                    tsl = slice(t * P, (t + 1) * P)
                    for d in range(NDCH):
                        dsl = slice(d * DCH, (d + 1) * DCH)
                        wi, doff = (0, d) if d < NDCH // 2 else (1, d - NDCH // 2)
                        w2sl = slice(doff * DCH, (doff + 1) * DCH)
                        py0 = pyp.tile([P, DCH], dt.float32, tag="py")
                        for i in range(IEO):
                            nc.tensor.matmul(py0, h0[:, i, tsl], w2a[wi][:, i, w2sl],
                                             start=(i == 0), stop=(i == IEO - 1))
                        py1 = pyp.tile([P, DCH], dt.float32, tag="py")
                        for i in range(IEO):
                            nc.tensor.matmul(py1, h1[:, i, tsl], w2b[wi][:, i, w2sl],
                                             start=(i == 0), stop=(i == IEO - 1))
                        pys = pyp.tile([P, DCH], dt.float32, tag="py")
                        for i in range(ISO):
                            nc.tensor.matmul(pys, hs[:, i, tsl], ws2_sb[:, i, dsl],
                                             start=(i == 0), stop=(i == ISO - 1))
                        y_t = ypool.tile([P, DCH], dt.float32, tag="yt")
                        nc.vector.tensor_scalar_mul(y_t, py0, coef_c[:, t, 0:1])
                        nc.vector.scalar_tensor_tensor(y_t, py1, coef_c[:, t, 1:2],
                                                       y_t, op0=ALU.mult, op1=ALU.add)
                        nc.vector.tensor_add(y_t, y_t, pys)
                        # y traffic rides the ACT HWDGE queue (see module doc)
                        nc.scalar.dma_start(y_part[c, tsl, dsl], y_t)

                    if c == n_chunks - 1:
                        # last chunk: reduce per token-slice to shrink the tail
                        nc.gpsimd.collective_compute(
                            "ReduceScatter",
                            ALU.add,
                            replica_groups=[list(range(N_CORES))],
                            ins=[y_part[c, tsl, :].opt()],
                            outs=[y_rs[c, t * (P // N_CORES):(t + 1) * (P // N_CORES), :].opt()],
                        )
                        nc.scalar.dma_start(
                            y_out[c, t * (P // N_CORES):(t + 1) * (P // N_CORES), :],
                            y_rs[c, t * (P // N_CORES):(t + 1) * (P // N_CORES), :])

                if c != n_chunks - 1:
                    # ---- reduce across cores; each core keeps its 64-row slice ----
                    nc.gpsimd.collective_compute(
                        "ReduceScatter",
                        ALU.add,
                        replica_groups=[list(range(N_CORES))],
                        ins=[y_part[c].opt()],
                        outs=[y_rs[c].opt()],
                    )
                    nc.scalar.dma_start(y_out[c], y_rs[c])

    nc.finalize()
    return nc


# ---------------- host-side data prep ----------------

def _x_layout(a, n_chunks):
    # [T, D] -> [n_chunks, P(ki), KO, CH]  (partition line = KO*CH contiguous)
    t_use = n_chunks * CH
    return np.ascontiguousarray(
        a[:t_use].reshape(n_chunks, CH, KO, P).transpose(0, 3, 2, 1))


def _lhs_layout(w):
    # [D, N] -> [P(ki), D//P(ko), N]
    d, n = w.shape
    return np.ascontiguousarray(w.reshape(d // P, P, n).transpose(1, 0, 2))


def _hilo(a):
    hi = a.astype(BF16)
    lo = (a - hi.astype(F32)).astype(BF16)
    return hi, lo


def make_in_maps(inputs, n_chunks=T // CH):
    x = np.asarray(inputs["x"], F32).reshape(T, D)
    gate_w = np.asarray(inputs["gate_w"], F32)
    w1 = np.asarray(inputs["w1"], F32)
    w2 = np.asarray(inputs["w2"], F32)
    w3 = np.asarray(inputs["w3"], F32)
    ws1 = np.asarray(inputs["ws1"], F32)
    ws2 = np.asarray(inputs["ws2"], F32)
    ws3 = np.asarray(inputs["ws3"], F32)

    xh, xl = _hilo(x)
    xh_t = _x_layout(xh, n_chunks)
    xl_t = _x_layout(xl, n_chunks)

    in_maps = []
    for core in range(N_CORES):
        ea, eb = 2 * core, 2 * core + 1
        cols = slice(core * SH_PC, (core + 1) * SH_PC)
        ws13 = np.concatenate([ws1[:, cols], ws3[:, cols]], axis=1)

        perm = [ea, eb] + [e for e in range(E) if e not in (ea, eb)]
        gp = gate_w[:, perm]
        gh, gl = _hilo(gp)
        z = np.zeros_like(gh)
        g1 = np.concatenate([gh, z, gl], axis=1)
        g2 = np.concatenate([z, z, gh], axis=1)

        in_maps.append({
            "xh": xh_t, "xl": xl_t,
            "w1a": _lhs_layout(w1[ea].astype(BF16)),
            "w3a": _lhs_layout(w3[ea].astype(BF16)),
            "w2a": _lhs_layout(w2[ea].astype(BF16)),
            "w1b": _lhs_layout(w1[eb].astype(BF16)),
            "w3b": _lhs_layout(w3[eb].astype(BF16)),
            "w2b": _lhs_layout(w2[eb].astype(BF16)),
            "ws13": _lhs_layout(ws13.astype(BF16)),
            "ws2": _lhs_layout(ws2[cols].astype(BF16)),
            "g1": _lhs_layout(g1),
            "g2": _lhs_layout(g2),
        })
    return in_maps


def assemble_output(results, n_chunks=T // CH):
    y = np.zeros((n_chunks * CH, D), F32)
    c_last = n_chunks - 1
    rs_t = P // N_CORES  # 16 rows per core per token-slice in the last chunk
    for core in range(N_CORES):
        r = np.asarray(results[core]["y_out"]).reshape(n_chunks, RS_OUT, D)
        for c in range(n_chunks):
            if c == c_last:
                for t in range(NTS):
                    dst = c * CH + t * P + core * rs_t
                    y[dst:dst + rs_t] = r[c, t * rs_t:(t + 1) * rs_t]
            else:
                y[c * CH + core * RS_OUT:(c * CH + (core + 1) * RS_OUT)] = r[c]
    return y


_NC_CACHE = {}


def kernel(**inputs) -> np.ndarray:
    from concourse.bass_utils import run_bass_kernel_spmd

    n_chunks = T // CH
    if n_chunks not in _NC_CACHE:
        _NC_CACHE[n_chunks] = build_nc(n_chunks)
    nc = _NC_CACHE[n_chunks]

    in_maps = make_in_maps(inputs, n_chunks)
    res = run_bass_kernel_spmd(nc, in_maps, core_ids=list(range(N_CORES)))
    y = assemble_output(res.results, n_chunks)
    return y.reshape(B, S, D)

